# revision 5
# baseline (speedup 1.0000x reference)
"""Multi-head causal attention (B=4, T=2048, D=1024, H=16, d_k=64) on 8 trn2 cores.

Sharding: 8 cores = 4 batches x 2 head-groups (8 heads each).
Per core: Q^T/K^T projections in [c, t] layout (contraction on partitions),
V in natural [s, c] layout with an appended ones column per head so the
attn@V matmul accumulates both context^T and the softmax denominator Z.
Scores are computed transposed (scores^T[s, t]) so softmax normalization is a
per-free-element broadcast done via a tiny ones-matmul; causal masking uses
gpsimd affine_select on diagonal blocks only (upper blocks are skipped).
Host pre-transposes x and sums the two head-group partial outputs per batch.
"""

import sys

if "/opt/trn_rl_repo" not in sys.path:
    sys.path.insert(0, "/opt/trn_rl_repo")

from contextlib import ExitStack

import numpy as np

import concourse.bacc as bacc
import concourse.mybir as mybir
import concourse.tile as tile
from concourse.bass_utils import run_bass_kernel_spmd

D = 1024  # model dim
C = 512   # per-core projection cols (8 heads x 64)
NJ = 4    # head-pair chunks of 128 channels
NKC = 8   # contraction chunks of 128 over D
DT = mybir.dt.float32
FR = mybir.dt.float32r
BF = mybir.dt.bfloat16


def build_nc(T=2048, loop_reps=1):
    """Build the per-core SPMD Bass program (identical on all cores).

    loop_reps>1 wraps the whole body in a hardware loop (timing builds only).
    """
    NR = T // 512   # t-ranges of 512
    NS = T // 128   # s-tiles of 128

    nc = bacc.Bacc("TRN2", target_bir_lowering=False, debug=False)
    xT_d = nc.dram_tensor("xT", [D, T], FR, kind="ExternalInput").ap()
    wq_d = nc.dram_tensor("wq", [D, C], FR, kind="ExternalInput").ap()
    wk_d = nc.dram_tensor("wk", [D, C], FR, kind="ExternalInput").ap()
    wv_d = nc.dram_tensor("wv", [D, C], FR, kind="ExternalInput").ap()
    wo_d = nc.dram_tensor("wo", [C, D], FR, kind="ExternalInput").ap()
    ones_d = nc.dram_tensor("ones", [1, 64], FR, kind="ExternalInput").ap()
    out_d = nc.dram_tensor("out", [T, D], DT, kind="ExternalOutput").ap()

    with tile.TileContext(nc) as tc, ExitStack() as ctx:
        if loop_reps > 1:
            ctx.enter_context(tc.For_i(0, loop_reps, 1))
        main = ctx.enter_context(tc.tile_pool(name="main", bufs=1))
        qt = main.tile([128, NJ, T], FR)        # Q^T: row p of chunk j = channel j*128+p
        kt = main.tile([128, NJ, T], FR)        # K^T
        vt = main.tile([128, NS, 8, 65], BF)    # V[s, head, 0:64] + ones col 64
        cx = main.tile([128, NJ, T], FR)        # context^T (normalized)

        # ---------------- Phase A: projections ----------------
        with tc.tile_pool(name="wpool", bufs=1) as wp, \
             tc.tile_pool(name="xpool", bufs=3) as xp, \
             tc.tile_pool(name="pproj", bufs=4, space="PSUM") as pp:
            wq_s = wp.tile([128, NKC, C], FR, tag="wq")
            wk_s = wp.tile([128, NKC, C], FR, tag="wk")
            wv_s = wp.tile([128, NKC, C], FR, tag="wv")
            nc.sync.dma_start(wq_s[:], wq_d.rearrange("(kc p) c -> p kc c", p=128))
            nc.sync.dma_start(wk_s[:], wk_d.rearrange("(kc p) c -> p kc c", p=128))
            nc.sync.dma_start(wv_s[:], wv_d.rearrange("(kc p) c -> p kc c", p=128))
            nc.vector.memset(vt[:, :, :, 64:65], 1.0)

            xT_v = xT_d.rearrange("(kc p) t -> p kc t", p=128)
            for r in range(NR):
                halves = []
                for hf in range(2):
                    xh = xp.tile([128, 4, 512], FR, tag="xt")
                    nc.sync.dma_start(
                        xh[:], xT_v[:, hf * 4:(hf + 1) * 4, r * 512:(r + 1) * 512])
                    halves.append(xh)

                def xchunk(kc):
                    return halves[kc // 4][:, kc % 4, :]

                for w_s, dst in ((wq_s, qt), (wk_s, kt)):
                    for j in range(NJ):
                        ps = pp.tile([128, 512], DT, tag="pp")
                        for kc in range(NKC):
                            nc.tensor.matmul(
                                ps[:], w_s[:, kc, j * 128:(j + 1) * 128], xchunk(kc),
                                start=(kc == 0), stop=(kc == NKC - 1))
                        nc.vector.tensor_copy(dst[:, j, r * 512:(r + 1) * 512], ps[:])
                for al in range(4):
                    a = r * 4 + al
                    ps = pp.tile([128, 512], DT, tag="pp")
                    for kc in range(NKC):
                        nc.tensor.matmul(
                            ps[:], xchunk(kc)[:, al * 128:(al + 1) * 128],
                            wv_s[:, kc, :],
                            start=(kc == 0), stop=(kc == NKC - 1))
                    nc.vector.tensor_copy(
                        vt[:, a, :, 0:64], ps[:].rearrange("p (h e) -> p h e", h=8))

        # ---------------- Phase B: attention ----------------
        with tc.tile_pool(name="const", bufs=1) as cp, \
             tc.tile_pool(name="epool", bufs=6) as ep, \
             tc.tile_pool(name="zpool", bufs=4) as zp, \
             tc.tile_pool(name="psS", bufs=4, space="PSUM") as pS, \
             tc.tile_pool(name="psC", bufs=2, space="PSUM") as pC, \
             tc.tile_pool(name="psB", bufs=2, space="PSUM") as pB:
            ones164 = cp.tile([1, 64], FR)
            nc.sync.dma_start(ones164[:], ones_d[:])
            for b in range(NR):
                na = 4 * b + 4  # s-tiles 0..4b+3 (rest masked out entirely)
                for j in range(NJ):
                    psc = [pC.tile([65, 512], DT, tag="psc", name=f"psc{b}_{j}_{h}")
                           for h in range(2)]
                    for a in range(na):
                        diag = a >= 4 * b
                        for h in range(2):
                            pss = pS.tile([128, 512], DT, tag="pss")
                            nc.tensor.matmul(
                                pss[:],
                                kt[h * 64:(h + 1) * 64, j, a * 128:(a + 1) * 128],
                                qt[h * 64:(h + 1) * 64, j, b * 512:(b + 1) * 512],
                                start=True, stop=True, tile_position=(h * 64, 0))
                            et = ep.tile([128, 512], BF, tag="et")
                            nc.scalar.activation(
                                et[:], pss[:], mybir.ActivationFunctionType.Exp,
                                scale=0.125)
                            if diag:
                                # keep where s <= t: f - p + (512b - 128a) >= 0
                                nc.gpsimd.affine_select(
                                    out=et[:], in_=et[:],
                                    compare_op=mybir.AluOpType.is_ge,
                                    fill=0.0, base=512 * b - 128 * a,
                                    pattern=[[1, 512]], channel_multiplier=-1)
                            nc.tensor.matmul(
                                psc[h][:], vt[:, a, 2 * j + h, :], et[:],
                                start=(a == 0), stop=(a == na - 1))
                    for h in range(2):
                        zr = zp.tile([1, 512], FR, tag="zr")
                        with nc.allow_low_precision(reason="f32r is fp32-width"):
                            nc.vector.reciprocal(zr[:], psc[h][64:65, :])
                        psb = pB.tile([64, 512], DT, tag="psb")
                        nc.tensor.matmul(psb[:], ones164[:], zr[:],
                                         start=True, stop=True)
                        zb = zp.tile([64, 512], DT, tag="zb")
                        nc.vector.tensor_copy(zb[:], psb[:])
                        nc.vector.tensor_mul(
                            cx[h * 64:(h + 1) * 64, j, b * 512:(b + 1) * 512],
                            psc[h][0:64, :], zb[:])

        # ---------------- Phase C: output projection ----------------
        with tc.tile_pool(name="wopool", bufs=1) as wop, \
             tc.tile_pool(name="opool", bufs=3) as op, \
             tc.tile_pool(name="psO", bufs=4, space="PSUM") as pO:
            wo_s = wop.tile([128, NJ, D], FR)
            nc.sync.dma_start(wo_s[:], wo_d.rearrange("(j p) o -> p j o", p=128))
            for ts in range(T // 128):
                ot = op.tile([128, D], DT, tag="ot")
                for oh in range(2):
                    ps = pO.tile([128, 512], DT, tag="pso")
                    for j in range(NJ):
                        nc.tensor.matmul(
                            ps[:], cx[:, j, ts * 128:(ts + 1) * 128],
                            wo_s[:, j, oh * 512:(oh + 1) * 512],
                            start=(j == 0), stop=(j == NJ - 1))
                    nc.vector.tensor_copy(ot[:, oh * 512:(oh + 1) * 512], ps[:])
                nc.sync.dma_start(out_d[ts * 128:(ts + 1) * 128, :], ot[:])

    nc.compile()
    return nc


def make_in_maps(x, W_q, W_k, W_v, W_o):
    in_maps = []
    for core in range(8):
        b, g = core // 2, core % 2
        sl = slice(g * C, (g + 1) * C)
        in_maps.append({
            "xT": np.ascontiguousarray(x[b].T).astype(np.float32),
            "wq": np.ascontiguousarray(W_q[:, sl]).astype(np.float32),
            "wk": np.ascontiguousarray(W_k[:, sl]).astype(np.float32),
            "wv": np.ascontiguousarray(W_v[:, sl]).astype(np.float32),
            "wo": np.ascontiguousarray(W_o[sl, :]).astype(np.float32),
            "ones": np.ones((1, 64), np.float32),
        })
    return in_maps


_NC_CACHE = {}


def kernel(x, W_q, W_k, W_v, W_o):
    x = np.asarray(x, dtype=np.float32)
    W_q = np.asarray(W_q, dtype=np.float32)
    W_k = np.asarray(W_k, dtype=np.float32)
    W_v = np.asarray(W_v, dtype=np.float32)
    W_o = np.asarray(W_o, dtype=np.float32)
    T = x.shape[1]
    if T not in _NC_CACHE:
        _NC_CACHE[T] = build_nc(T)
    nc = _NC_CACHE[T]
    res = run_bass_kernel_spmd(nc, make_in_maps(x, W_q, W_k, W_v, W_o),
                               list(range(8))).results
    out = np.stack([res[2 * b]["out"] + res[2 * b + 1]["out"] for b in range(4)])
    return out.astype(np.float32)


# revision 6
# speedup vs baseline: 1.0199x; 1.0199x over previous
"""Multi-head causal attention (B=4, T=2048, D=1024, H=16, d_k=64) on 8 trn2 cores.

Sharding: 8 cores = 4 batches x 2 head-groups (8 heads each).
Per core: Q^T/K^T projections in [c, t] layout (contraction on partitions),
V in natural [s, c] layout with an appended ones column per head so the
attn@V matmul accumulates both context^T and the softmax denominator Z.
Scores are computed transposed (scores^T[s, t]) so softmax normalization is a
per-free-element broadcast done via a tiny ones-matmul; causal masking uses
gpsimd affine_select on diagonal blocks only (upper blocks are skipped).
Matmuls run in float32r (4x fp32 PE throughput, ~1e-4 matmul error); E/V are
bf16. Phases are interleaved per 512-row t-range and the attn@V matmul is
software-pipelined 2 tiles behind the scores matmul so the PE never waits on
the exp/mask chain. Host pre-transposes x and sums the two head-group partial
outputs per batch.
"""

import sys

if "/opt/trn_rl_repo" not in sys.path:
    sys.path.insert(0, "/opt/trn_rl_repo")

from contextlib import ExitStack

import numpy as np

import concourse.bacc as bacc
import concourse.mybir as mybir
import concourse.tile as tile
from concourse.bass_utils import run_bass_kernel_spmd

D = 1024  # model dim
C = 512   # per-core projection cols (8 heads x 64)
NJ = 4    # head-pair chunks of 128 channels
NKC = 8   # contraction chunks of 128 over D
DT = mybir.dt.float32
FR = mybir.dt.float32r
BF = mybir.dt.bfloat16
EXP = mybir.ActivationFunctionType.Exp


def build_nc(T=2048, loop_reps=1):
    """Build the per-core SPMD Bass program (identical on all cores).

    loop_reps>1 wraps the whole body in a hardware loop (timing builds only).
    """
    NR = T // 512   # t-ranges of 512

    nc = bacc.Bacc("TRN2", target_bir_lowering=False, debug=False)
    xT_d = nc.dram_tensor("xT", [D, T], FR, kind="ExternalInput").ap()
    wq_d = nc.dram_tensor("wq", [D, C], FR, kind="ExternalInput").ap()
    wk_d = nc.dram_tensor("wk", [D, C], FR, kind="ExternalInput").ap()
    wv_d = nc.dram_tensor("wv", [D, C], FR, kind="ExternalInput").ap()
    wo_d = nc.dram_tensor("wo", [C, D], FR, kind="ExternalInput").ap()
    ones_d = nc.dram_tensor("ones", [1, 64], FR, kind="ExternalInput").ap()
    out_d = nc.dram_tensor("out", [T, D], DT, kind="ExternalOutput").ap()

    with tile.TileContext(nc) as tc, ExitStack() as ctx:
        if loop_reps > 1:
            ctx.enter_context(tc.For_i(0, loop_reps, 1))
        main = ctx.enter_context(tc.tile_pool(name="main", bufs=1))
        qtp = ctx.enter_context(tc.tile_pool(name="qtp", bufs=2))
        cxp = ctx.enter_context(tc.tile_pool(name="cxp", bufs=2))
        xp = ctx.enter_context(tc.tile_pool(name="xp", bufs=3))
        ep = ctx.enter_context(tc.tile_pool(name="ep", bufs=6))
        zp = ctx.enter_context(tc.tile_pool(name="zp", bufs=4))
        op = ctx.enter_context(tc.tile_pool(name="op", bufs=2))
        pp = ctx.enter_context(tc.tile_pool(name="pp", bufs=2, space="PSUM"))
        pS = ctx.enter_context(tc.tile_pool(name="pS", bufs=3, space="PSUM"))
        pC = ctx.enter_context(tc.tile_pool(name="pC", bufs=2, space="PSUM"))
        pB = ctx.enter_context(tc.tile_pool(name="pB", bufs=1, space="PSUM"))

        wq_s = main.tile([128, NKC, C], FR, tag="wq")
        wk_s = main.tile([128, NKC, C], FR, tag="wk")
        wv_s = main.tile([128, NKC, C], FR, tag="wv")
        wo_s = main.tile([128, NJ, D], FR, tag="wo")
        ones164 = main.tile([1, 64], FR, tag="ones164")
        nc.sync.dma_start(wq_s[:], wq_d.rearrange("(kc p) c -> p kc c", p=128))
        nc.sync.dma_start(wk_s[:], wk_d.rearrange("(kc p) c -> p kc c", p=128))
        nc.sync.dma_start(wv_s[:], wv_d.rearrange("(kc p) c -> p kc c", p=128))
        nc.sync.dma_start(wo_s[:], wo_d.rearrange("(j p) o -> p j o", p=128))
        nc.sync.dma_start(ones164[:], ones_d[:])

        xT_v = xT_d.rearrange("(kc p) t -> p kc t", p=128)
        kts = []   # per-r K^T tiles [128, NJ, 512]
        vts = []   # per-r V tiles [128, 4, 8, 65] (s-tiles 4r..4r+3)

        for r in range(NR):
            # ---------------- projections for t/s-range r ----------------
            halves = []
            for hf in range(2):
                xh = xp.tile([128, 4, 512], FR, tag="xt", name=f"xh{r}_{hf}")
                nc.sync.dma_start(
                    xh[:], xT_v[:, hf * 4:(hf + 1) * 4, r * 512:(r + 1) * 512])
                halves.append(xh)

            def xchunk(kc):
                return halves[kc // 4][:, kc % 4, :]

            qt = qtp.tile([128, NJ, 512], FR, tag="qt", name=f"qt{r}")
            kt = main.tile([128, NJ, 512], FR, tag=f"kt{r}", name=f"kt{r}")
            vt = main.tile([128, 4, 8, 65], BF, tag=f"vt{r}", name=f"vt{r}")
            kts.append(kt)
            vts.append(vt)
            nc.vector.memset(vt[:, :, :, 64:65], 1.0)
            for w_s, dst in ((wq_s, qt), (wk_s, kt)):
                for j in range(NJ):
                    ps = pp.tile([128, 512], DT, tag="pp", name=f"psqk{r}_{j}")
                    for kc in range(NKC):
                        nc.tensor.matmul(
                            ps[:], w_s[:, kc, j * 128:(j + 1) * 128], xchunk(kc),
                            start=(kc == 0), stop=(kc == NKC - 1))
                    nc.vector.tensor_copy(dst[:, j, :], ps[:])
            for al in range(4):
                ps = pp.tile([128, 512], DT, tag="pp", name=f"psv{r}_{al}")
                for kc in range(NKC):
                    nc.tensor.matmul(
                        ps[:], xchunk(kc)[:, al * 128:(al + 1) * 128], wv_s[:, kc, :],
                        start=(kc == 0), stop=(kc == NKC - 1))
                nc.vector.tensor_copy(
                    vt[:, al, :, 0:64], ps[:].rearrange("p (h e) -> p h e", h=8))

            # ---------------- attention for t-range b = r ----------------
            b = r
            na = 4 * b + 4
            cx = cxp.tile([128, NJ, 512], FR, tag="cx", name=f"cx{r}")
            for j in range(NJ):
                psc = [pC.tile([65, 512], DT, tag="psc", name=f"psc{b}_{j}_{h}")
                       for h in range(2)]
                pend = []  # software pipeline: (a, h, et) awaiting attn@V

                def flush(n, psc=psc, pend=pend, j=j, na=na):
                    while len(pend) > n:
                        a0, h0, et0 = pend.pop(0)
                        nc.tensor.matmul(
                            psc[h0][:], vts[a0 // 4][:, a0 % 4, 2 * j + h0, :],
                            et0[:], start=(a0 == 0), stop=(a0 == na - 1))

                for a in range(na):
                    diag = a >= 4 * b
                    for h in range(2):
                        pss = pS.tile([128, 512], DT, tag="pss",
                                      name=f"pss{b}_{j}_{a}_{h}")
                        nc.tensor.matmul(
                            pss[:],
                            kts[a // 4][h * 64:(h + 1) * 64, j,
                                        (a % 4) * 128:(a % 4 + 1) * 128],
                            qt[h * 64:(h + 1) * 64, j, :],
                            start=True, stop=True, tile_position=(h * 64, 0))
                        et = ep.tile([128, 512], BF, tag="et",
                                     name=f"et{b}_{j}_{a}_{h}")
                        nc.scalar.activation(et[:], pss[:], EXP, scale=0.125)
                        if diag:
                            # keep where s <= t: f - p + (512b - 128a) >= 0
                            nc.gpsimd.affine_select(
                                out=et[:], in_=et[:],
                                compare_op=mybir.AluOpType.is_ge,
                                fill=0.0, base=512 * b - 128 * a,
                                pattern=[[1, 512]], channel_multiplier=-1)
                        pend.append((a, h, et))
                    flush(4)  # keep attn@V two (a, h)-tiles behind scores
                flush(0)
                for h in range(2):
                    zr = zp.tile([1, 512], FR, tag="zr", name=f"zr{b}_{j}_{h}")
                    with nc.allow_low_precision(reason="f32r is fp32-width"):
                        nc.vector.reciprocal(zr[:], psc[h][64:65, :])
                    psb = pB.tile([64, 512], DT, tag="psb", name=f"psb{b}_{j}_{h}")
                    nc.tensor.matmul(psb[:], ones164[:], zr[:], start=True, stop=True)
                    zb = zp.tile([64, 512], DT, tag="zb", name=f"zb{b}_{j}_{h}")
                    nc.vector.tensor_copy(zb[:], psb[:])
                    nc.vector.tensor_mul(
                        cx[h * 64:(h + 1) * 64, j, :], psc[h][0:64, :], zb[:])

            # ---------------- output projection for t-range r ----------------
            for ts in range(4):
                ot = op.tile([128, D], DT, tag="ot", name=f"ot{r}_{ts}")
                for oh in range(2):
                    ps = pp.tile([128, 512], DT, tag="pp", name=f"pso{r}_{ts}_{oh}")
                    for j in range(NJ):
                        nc.tensor.matmul(
                            ps[:], cx[:, j, ts * 128:(ts + 1) * 128],
                            wo_s[:, j, oh * 512:(oh + 1) * 512],
                            start=(j == 0), stop=(j == NJ - 1))
                    nc.vector.tensor_copy(ot[:, oh * 512:(oh + 1) * 512], ps[:])
                nc.sync.dma_start(
                    out_d[(r * 4 + ts) * 128:(r * 4 + ts + 1) * 128, :], ot[:])

    nc.compile()
    return nc


def make_in_maps(x, W_q, W_k, W_v, W_o):
    in_maps = []
    for core in range(8):
        b, g = core // 2, core % 2
        sl = slice(g * C, (g + 1) * C)
        in_maps.append({
            "xT": np.ascontiguousarray(x[b].T).astype(np.float32),
            "wq": np.ascontiguousarray(W_q[:, sl]).astype(np.float32),
            "wk": np.ascontiguousarray(W_k[:, sl]).astype(np.float32),
            "wv": np.ascontiguousarray(W_v[:, sl]).astype(np.float32),
            "wo": np.ascontiguousarray(W_o[sl, :]).astype(np.float32),
            "ones": np.ones((1, 64), np.float32),
        })
    return in_maps


_NC_CACHE = {}


def kernel(x, W_q, W_k, W_v, W_o):
    x = np.asarray(x, dtype=np.float32)
    W_q = np.asarray(W_q, dtype=np.float32)
    W_k = np.asarray(W_k, dtype=np.float32)
    W_v = np.asarray(W_v, dtype=np.float32)
    W_o = np.asarray(W_o, dtype=np.float32)
    T = x.shape[1]
    if T not in _NC_CACHE:
        _NC_CACHE[T] = build_nc(T)
    nc = _NC_CACHE[T]
    res = run_bass_kernel_spmd(nc, make_in_maps(x, W_q, W_k, W_v, W_o),
                               list(range(8))).results
    out = np.stack([res[2 * b]["out"] + res[2 * b + 1]["out"] for b in range(4)])
    return out.astype(np.float32)


# revision 7
# speedup vs baseline: 1.1124x; 1.0907x over previous
"""Multi-head causal attention (B=4, T=2048, D=1024, H=16, d_k=64) on 8 trn2 cores.

Sharding: 8 cores = 4 batches x 2 head-groups (8 heads each).
Per core: Q^T/K^T projections in [c, t] layout (contraction on partitions),
V in natural [s, c] layout with an appended ones column per head so the
attn@V matmul accumulates both context^T and the softmax denominator Z.
Scores are computed transposed (scores^T[s, t]) so softmax normalization is a
per-free-element broadcast done via a tiny ones-matmul; causal masking uses
gpsimd affine_select on diagonal blocks only (upper blocks are skipped).
Matmuls run in float32r (4x fp32 PE throughput, ~1e-4 matmul error); E/V are
bf16. Phases are interleaved per 512-row t-range and the attn@V matmul is
software-pipelined 2 tiles behind the scores matmul so the PE never waits on
the exp/mask chain. Host pre-transposes x and sums the two head-group partial
outputs per batch.
"""

import sys

if "/opt/trn_rl_repo" not in sys.path:
    sys.path.insert(0, "/opt/trn_rl_repo")

from contextlib import ExitStack

import ml_dtypes
import numpy as np

import concourse.bacc as bacc
import concourse.mybir as mybir
import concourse.tile as tile
from concourse.bass_utils import run_bass_kernel_spmd

D = 1024  # model dim
C = 512   # per-core projection cols (8 heads x 64)
NJ = 4    # head-pair chunks of 128 channels
NKC = 8   # contraction chunks of 128 over D
DT = mybir.dt.float32
FR = mybir.dt.float32r
BF = mybir.dt.bfloat16
EXP = mybir.ActivationFunctionType.Exp


def build_nc(T=2048, loop_reps=1):
    """Build the per-core SPMD Bass program (identical on all cores).

    loop_reps>1 wraps the whole body in a hardware loop (timing builds only).
    """
    NR = T // 512   # t-ranges of 512

    nc = bacc.Bacc("TRN2", target_bir_lowering=False, debug=False)
    xT_d = nc.dram_tensor("xT", [D, T], BF, kind="ExternalInput").ap()
    wq_d = nc.dram_tensor("wq", [D, C], BF, kind="ExternalInput").ap()
    wk_d = nc.dram_tensor("wk", [D, C], BF, kind="ExternalInput").ap()
    wv_d = nc.dram_tensor("wv", [D, C], BF, kind="ExternalInput").ap()
    wo_d = nc.dram_tensor("wo", [C, D], BF, kind="ExternalInput").ap()
    ones_d = nc.dram_tensor("ones", [1, 64], FR, kind="ExternalInput").ap()
    out_d = nc.dram_tensor("out", [T, D], DT, kind="ExternalOutput").ap()

    with tile.TileContext(nc) as tc, ExitStack() as ctx:
        if loop_reps > 1:
            ctx.enter_context(tc.For_i(0, loop_reps, 1))
        main = ctx.enter_context(tc.tile_pool(name="main", bufs=1))
        qtp = ctx.enter_context(tc.tile_pool(name="qtp", bufs=2))
        cxp = ctx.enter_context(tc.tile_pool(name="cxp", bufs=2))
        xp = ctx.enter_context(tc.tile_pool(name="xp", bufs=3))
        ep = ctx.enter_context(tc.tile_pool(name="ep", bufs=6))
        zp = ctx.enter_context(tc.tile_pool(name="zp", bufs=4))
        op = ctx.enter_context(tc.tile_pool(name="op", bufs=2))
        pp = ctx.enter_context(tc.tile_pool(name="pp", bufs=2, space="PSUM"))
        pS = ctx.enter_context(tc.tile_pool(name="pS", bufs=3, space="PSUM"))
        pC = ctx.enter_context(tc.tile_pool(name="pC", bufs=2, space="PSUM"))
        pB = ctx.enter_context(tc.tile_pool(name="pB", bufs=1, space="PSUM"))

        wq_s = main.tile([128, NKC, C], BF, tag="wq")
        wk_s = main.tile([128, NKC, C], BF, tag="wk")
        wv_s = main.tile([128, NKC, C], BF, tag="wv")
        wo_s = main.tile([128, NJ, D], BF, tag="wo")
        ones164 = main.tile([1, 64], FR, tag="ones164")
        nc.sync.dma_start(wq_s[:], wq_d.rearrange("(kc p) c -> p kc c", p=128))
        nc.sync.dma_start(wk_s[:], wk_d.rearrange("(kc p) c -> p kc c", p=128))
        nc.sync.dma_start(wv_s[:], wv_d.rearrange("(kc p) c -> p kc c", p=128))
        nc.sync.dma_start(wo_s[:], wo_d.rearrange("(j p) o -> p j o", p=128))
        nc.sync.dma_start(ones164[:], ones_d[:])

        xT_v = xT_d.rearrange("(kc p) t -> p kc t", p=128)
        kts = []   # per-r K^T tiles [128, NJ, 512]
        vts = []   # per-r V tiles [128, 4, 8, 65] (s-tiles 4r..4r+3)

        for r in range(NR):
            # ---------------- projections for t/s-range r ----------------
            halves = []
            for hf in range(2):
                xh = xp.tile([128, 4, 512], BF, tag="xt", name=f"xh{r}_{hf}")
                nc.sync.dma_start(
                    xh[:], xT_v[:, hf * 4:(hf + 1) * 4, r * 512:(r + 1) * 512])
                halves.append(xh)

            def xchunk(kc):
                return halves[kc // 4][:, kc % 4, :]

            qt = qtp.tile([128, NJ, 512], BF, tag="qt", name=f"qt{r}")
            kt = main.tile([128, NJ, 512], BF, tag=f"kt{r}", name=f"kt{r}")
            vt = main.tile([128, 4, 8, 65], BF, tag=f"vt{r}", name=f"vt{r}")
            kts.append(kt)
            vts.append(vt)
            nc.vector.memset(vt[:, :, :, 64:65], 1.0)
            for w_s, dst in ((wq_s, qt), (wk_s, kt)):
                for j in range(NJ):
                    ps = pp.tile([128, 512], DT, tag="pp", name=f"psqk{r}_{j}")
                    for kc in range(NKC):
                        nc.tensor.matmul(
                            ps[:], w_s[:, kc, j * 128:(j + 1) * 128], xchunk(kc),
                            start=(kc == 0), stop=(kc == NKC - 1))
                    with nc.allow_low_precision(reason="bf16 store"):
                        nc.vector.tensor_copy(dst[:, j, :], ps[:])
            for al in range(4):
                ps = pp.tile([128, 512], DT, tag="pp", name=f"psv{r}_{al}")
                for kc in range(NKC):
                    nc.tensor.matmul(
                        ps[:], xchunk(kc)[:, al * 128:(al + 1) * 128], wv_s[:, kc, :],
                        start=(kc == 0), stop=(kc == NKC - 1))
                with nc.allow_low_precision(reason="bf16 store"):
                    nc.vector.tensor_copy(
                        vt[:, al, :, 0:64], ps[:].rearrange("p (h e) -> p h e", h=8))

            # ---------------- attention for t-range b = r ----------------
            b = r
            na = 4 * b + 4
            cx = cxp.tile([128, NJ, 512], BF, tag="cx", name=f"cx{r}")
            for j in range(NJ):
                psc = [pC.tile([65, 512], DT, tag="psc", name=f"psc{b}_{j}_{h}")
                       for h in range(2)]
                pend = []  # software pipeline: (a, h, et) awaiting attn@V

                def flush(n, psc=psc, pend=pend, j=j, na=na):
                    while len(pend) > n:
                        a0, h0, et0 = pend.pop(0)
                        nc.tensor.matmul(
                            psc[h0][:], vts[a0 // 4][:, a0 % 4, 2 * j + h0, :],
                            et0[:], start=(a0 == 0), stop=(a0 == na - 1))

                for a in range(na):
                    diag = a >= 4 * b
                    for h in range(2):
                        pss = pS.tile([128, 512], DT, tag="pss",
                                      name=f"pss{b}_{j}_{a}_{h}")
                        nc.tensor.matmul(
                            pss[:],
                            kts[a // 4][h * 64:(h + 1) * 64, j,
                                        (a % 4) * 128:(a % 4 + 1) * 128],
                            qt[h * 64:(h + 1) * 64, j, :],
                            start=True, stop=True, tile_position=(h * 64, 0))
                        et = ep.tile([128, 512], BF, tag="et",
                                     name=f"et{b}_{j}_{a}_{h}")
                        nc.scalar.activation(et[:], pss[:], EXP, scale=0.125)
                        if diag:
                            # keep where s <= t: f - p + (512b - 128a) >= 0
                            nc.gpsimd.affine_select(
                                out=et[:], in_=et[:],
                                compare_op=mybir.AluOpType.is_ge,
                                fill=0.0, base=512 * b - 128 * a,
                                pattern=[[1, 512]], channel_multiplier=-1)
                        pend.append((a, h, et))
                    flush(4)  # keep attn@V two (a, h)-tiles behind scores
                flush(0)
                for h in range(2):
                    zr = zp.tile([1, 512], FR, tag="zr", name=f"zr{b}_{j}_{h}")
                    with nc.allow_low_precision(reason="f32r is fp32-width"):
                        nc.vector.reciprocal(zr[:], psc[h][64:65, :])
                    psb = pB.tile([64, 512], DT, tag="psb", name=f"psb{b}_{j}_{h}")
                    nc.tensor.matmul(psb[:], ones164[:], zr[:], start=True, stop=True)
                    zb = zp.tile([64, 512], DT, tag="zb", name=f"zb{b}_{j}_{h}")
                    nc.vector.tensor_copy(zb[:], psb[:])
                    with nc.allow_low_precision(reason="bf16 store"):
                        nc.vector.tensor_mul(
                            cx[h * 64:(h + 1) * 64, j, :], psc[h][0:64, :], zb[:])

            # ---------------- output projection for t-range r ----------------
            for ts in range(4):
                ot = op.tile([128, D], DT, tag="ot", name=f"ot{r}_{ts}")
                for oh in range(2):
                    ps = pp.tile([128, 512], DT, tag="pp", name=f"pso{r}_{ts}_{oh}")
                    for j in range(NJ):
                        nc.tensor.matmul(
                            ps[:], cx[:, j, ts * 128:(ts + 1) * 128],
                            wo_s[:, j, oh * 512:(oh + 1) * 512],
                            start=(j == 0), stop=(j == NJ - 1))
                    nc.vector.tensor_copy(ot[:, oh * 512:(oh + 1) * 512], ps[:])
                nc.sync.dma_start(
                    out_d[(r * 4 + ts) * 128:(r * 4 + ts + 1) * 128, :], ot[:])

    nc.compile()
    return nc


def make_in_maps(x, W_q, W_k, W_v, W_o):
    in_maps = []
    for core in range(8):
        b, g = core // 2, core % 2
        sl = slice(g * C, (g + 1) * C)
        in_maps.append({
            "xT": np.ascontiguousarray(x[b].T).astype(ml_dtypes.bfloat16),
            "wq": np.ascontiguousarray(W_q[:, sl]).astype(ml_dtypes.bfloat16),
            "wk": np.ascontiguousarray(W_k[:, sl]).astype(ml_dtypes.bfloat16),
            "wv": np.ascontiguousarray(W_v[:, sl]).astype(ml_dtypes.bfloat16),
            "wo": np.ascontiguousarray(W_o[sl, :]).astype(ml_dtypes.bfloat16),
            "ones": np.ones((1, 64), np.float32),
        })
    return in_maps


_NC_CACHE = {}


def kernel(x, W_q, W_k, W_v, W_o):
    x = np.asarray(x, dtype=np.float32)
    W_q = np.asarray(W_q, dtype=np.float32)
    W_k = np.asarray(W_k, dtype=np.float32)
    W_v = np.asarray(W_v, dtype=np.float32)
    W_o = np.asarray(W_o, dtype=np.float32)
    T = x.shape[1]
    if T not in _NC_CACHE:
        _NC_CACHE[T] = build_nc(T)
    nc = _NC_CACHE[T]
    res = run_bass_kernel_spmd(nc, make_in_maps(x, W_q, W_k, W_v, W_o),
                               list(range(8))).results
    out = np.stack([res[2 * b]["out"] + res[2 * b + 1]["out"] for b in range(4)])
    return out.astype(np.float32)
